# revision 10
# baseline (speedup 1.0000x reference)
"""Trainium2 Bass kernel for single-head attention (no mask).

Reference computation (B=4, S=2048, D=1024):
    q = x @ Wq.T ; k = x @ Wk.T ; v = x @ Wv.T          (per batch)
    out = softmax((q @ k.T) / sqrt(1024)) @ v

Sharding: 8 cores = (batch, query-half). Each core receives its batch's
x with its own query half reordered to the front (attention output is
invariant to a common permutation of the key/value rows), computes K/V
for the full sequence and Q for its 1024 rows, and writes its half of
the output. No collectives.

Scores are computed transposed (S^T[kpos, q]) so that softmax
normalization can use a ones-matmul for the kpos-sum and the PV matmul
consumes exp(S^T) tiles directly as the stationary operand, yielding
the output in natural [q, o] layout.

Matmul operands use float32r (full fp32 data, single-pass PE mode).
"""

import numpy as np

import concourse.bass as bass
import concourse.tile as tile
from concourse import bacc, mybir
from concourse.bass_utils import run_bass_kernel_spmd

B, S, D = 4, 2048, 1024
O = 1024  # d_out of each projection
HALF = S // 2  # query rows per core
N_CORES = 8
DT = mybir.dt.float32r
F32 = mybir.dt.float32
SCALE = 1.0 / 32.0  # 1/sqrt(1024)

SB = 256  # s-block (columns of x^T handled per projection block)
NBLK = S // SB  # 8
NQBLK = HALF // SB  # 4 blocks that also produce Q^T
DK = D // 128  # 8 contraction tiles
NOT = O // 128  # 8 output o-tiles
NKT = S // 128  # 16 kpos tiles

_CACHE: dict = {}


def _emit(nc):
    xf = nc.dram_tensor("xf", [S, D], DT, kind="ExternalInput")
    wq = nc.dram_tensor("wq", [O, D], DT, kind="ExternalInput")
    wk = nc.dram_tensor("wk", [O, D], DT, kind="ExternalInput")
    wv = nc.dram_tensor("wv", [O, D], DT, kind="ExternalInput")
    ident_in = nc.dram_tensor("ident", [128, 128], DT, kind="ExternalInput")
    ones_in = nc.dram_tensor("ones_in", [128, 128], DT, kind="ExternalInput")
    out = nc.dram_tensor("out", [HALF, O], F32, kind="ExternalOutput")
    kt_s = nc.dram_tensor("kt_s", [O, S], DT)  # K^T scratch
    v_s = nc.dram_tensor("v_s", [S, O], DT)  # V scratch

    with tile.TileContext(nc) as tc:
        with (
            tc.tile_pool(name="const", bufs=1) as constp,
            tc.tile_pool(name="persist", bufs=1) as persist,
        ):
            ident = constp.tile([128, 128], DT, tag="ident")
            nc.sync.dma_start(out=ident, in_=ident_in[:, :])
            ones = constp.tile([128, 128], DT, tag="ones")
            nc.sync.dma_start(out=ones, in_=ones_in[:, :])

            qt = persist.tile([128, NOT, HALF], DT, tag="qt")  # Q^T [o, q]

            # ---------- P0 + P1: projections ----------
            with (
                tc.tile_pool(name="wt", bufs=1) as wtp,
                tc.tile_pool(name="wnat", bufs=3) as wnat,
                tc.tile_pool(name="xin", bufs=3) as xin,
                tc.tile_pool(name="xt", bufs=2) as xtp,
                tc.tile_pool(name="kqev", bufs=3) as kqev,
                tc.tile_pool(name="vev", bufs=3) as vev,
                tc.tile_pool(name="tp_ps", bufs=2, space="PSUM") as tp_ps,
                tc.tile_pool(name="kq_ps", bufs=3, space="PSUM") as kq_ps,
                tc.tile_pool(name="v_ps", bufs=2, space="PSUM") as v_ps,
            ):
                # W^T tiles: [128(d within tile), DK index, O]
                wts = {}
                for name, wsrc in (("wtk", wk), ("wtv", wv), ("wtq", wq)):
                    wt = wtp.tile([128, DK, O], DT, tag=name, name=name)
                    wts[name] = wt
                    for ot in range(NOT):
                        wn = wnat.tile([128, D], DT, tag="wnat")
                        nc.sync.dma_start(
                            out=wn, in_=wsrc[ot * 128 : (ot + 1) * 128, :]
                        )
                        for dk in range(0, DK, 2):
                            ps = tp_ps.tile([128, 256], DT, tag="tp")
                            nc.tensor.transpose(
                                ps[:, 0:128], wn[:, dk * 128 : (dk + 1) * 128], ident
                            )
                            nc.tensor.transpose(
                                ps[:, 128:256],
                                wn[:, (dk + 1) * 128 : (dk + 2) * 128],
                                ident,
                            )
                            nc.vector.tensor_copy(
                                out=wt[:, dk, ot * 128 : (ot + 1) * 128], in_=ps[:, 0:128]
                            )
                            nc.vector.tensor_copy(
                                out=wt[:, dk + 1, ot * 128 : (ot + 1) * 128],
                                in_=ps[:, 128:256],
                            )

                wtk, wtv, wtq = wts["wtk"], wts["wtv"], wts["wtq"]

                for blk in range(NBLK):
                    # load 2 natural x tiles, transpose to xt_blk [128, DK, SB]
                    xt_blk = xtp.tile([128, DK, SB], DT, tag="xt")
                    xtiles = []
                    for st in range(2):
                        xn = xin.tile([128, D], DT, tag="xin")
                        nc.sync.dma_start(
                            out=xn,
                            in_=xf[blk * SB + st * 128 : blk * SB + (st + 1) * 128, :],
                        )
                        xtiles.append(xn)
                    for dk in range(DK):
                        ps = tp_ps.tile([128, 256], DT, tag="tp")
                        nc.tensor.transpose(
                            ps[:, 0:128], xtiles[0][:, dk * 128 : (dk + 1) * 128], ident
                        )
                        nc.tensor.transpose(
                            ps[:, 128:256], xtiles[1][:, dk * 128 : (dk + 1) * 128], ident
                        )
                        nc.scalar.copy(out=xt_blk[:, dk, :], in_=ps[:, :])

                    # K^T[:, blk]
                    for ot in range(NOT):
                        ps = kq_ps.tile([128, SB], F32, tag="kq")
                        for dk in range(DK):
                            nc.tensor.matmul(
                                ps[:, :],
                                wtk[:, dk, ot * 128 : (ot + 1) * 128],
                                xt_blk[:, dk, :],
                                start=(dk == 0),
                                stop=(dk == DK - 1),
                            )
                        ev = kqev.tile([128, SB], DT, tag="kqev")
                        nc.vector.tensor_copy(out=ev, in_=ps[:, :])
                        nc.gpsimd.dma_start(
                            out=kt_s[
                                ot * 128 : (ot + 1) * 128, blk * SB : (blk + 1) * SB
                            ],
                            in_=ev,
                        )

                    # V[blk, :]
                    for st in range(2):
                        for oc in range(2):
                            ps = v_ps.tile([128, 512], F32, tag="v")
                            for dk in range(DK):
                                nc.tensor.matmul(
                                    ps[:, :],
                                    xt_blk[:, dk, st * 128 : (st + 1) * 128],
                                    wtv[:, dk, oc * 512 : (oc + 1) * 512],
                                    start=(dk == 0),
                                    stop=(dk == DK - 1),
                                )
                            ev = vev.tile([128, 512], DT, tag="vev")
                            nc.vector.tensor_copy(out=ev, in_=ps[:, :])
                            nc.gpsimd.dma_start(
                                out=v_s[
                                    blk * SB + st * 128 : blk * SB + (st + 1) * 128,
                                    oc * 512 : (oc + 1) * 512,
                                ],
                                in_=ev,
                            )

                    # Q^T[:, blk] (first NQBLK blocks hold this core's queries)
                    if blk < NQBLK:
                        for ot in range(NOT):
                            ps = kq_ps.tile([128, SB], F32, tag="kq")
                            for dk in range(DK):
                                nc.tensor.matmul(
                                    ps[:, :],
                                    wtq[:, dk, ot * 128 : (ot + 1) * 128],
                                    xt_blk[:, dk, :],
                                    start=(dk == 0),
                                    stop=(dk == DK - 1),
                                )
                            nc.vector.tensor_copy(
                                out=qt[:, ot, blk * SB : (blk + 1) * SB], in_=ps[:, :]
                            )

            # ---------- P2: scores^T + exp + rowsums ----------
            with tc.tile_pool(name="et", bufs=1) as etp:
                et_tiles = [
                    etp.tile([128, HALF], DT, tag=f"et{i}", name=f"et{i}")
                    for i in range(NKT)
                ]
                with (
                    tc.tile_pool(name="ktin", bufs=3) as ktin,
                    tc.tile_pool(name="rsb", bufs=1) as rsb,
                    tc.tile_pool(name="s_ps", bufs=2, space="PSUM") as s_ps,
                    tc.tile_pool(name="rs_ps", bufs=1, space="PSUM") as rs_ps,
                ):
                    ps_rs = rs_ps.tile([128, HALF], F32, tag="rs")

                    for kt_i in range(NKT):
                        ktt = ktin.tile([128, NOT, 128], DT, tag="ktin")
                        nc.sync.dma_start(
                            out=ktt,
                            in_=kt_s[:, kt_i * 128 : (kt_i + 1) * 128].rearrange(
                                "(a p) f -> p a f", p=128
                            ),
                        )
                        ps = s_ps.tile([128, HALF], F32, tag="s")
                        for qc in range(2):
                            for ok in range(NOT):
                                nc.tensor.matmul(
                                    ps[:, qc * 512 : (qc + 1) * 512],
                                    ktt[:, ok, :],
                                    qt[:, ok, qc * 512 : (qc + 1) * 512],
                                    start=(ok == 0),
                                    stop=(ok == NOT - 1),
                                )
                        nc.scalar.activation(
                            out=et_tiles[kt_i],
                            in_=ps[:, :],
                            func=mybir.ActivationFunctionType.Exp,
                            scale=SCALE,
                        )
                        for qc in range(2):
                            nc.tensor.matmul(
                                ps_rs[:, qc * 512 : (qc + 1) * 512],
                                ones,
                                et_tiles[kt_i][:, qc * 512 : (qc + 1) * 512],
                                start=(kt_i == 0),
                                stop=(kt_i == NKT - 1),
                            )

                    recip = rsb.tile([128, HALF], F32, tag="recip")
                    nc.vector.reciprocal(out=recip, in_=ps_rs[:, :])
                    for kt_i in range(NKT):
                        nc.vector.tensor_mul(et_tiles[kt_i], et_tiles[kt_i], recip)

                # ---------- P3: out = (E^T)^T @ V ----------
                with (
                    tc.tile_pool(name="vin", bufs=4) as vin,
                    tc.tile_pool(name="oev", bufs=3) as oev,
                    tc.tile_pool(name="o_ps", bufs=8, space="PSUM") as o_ps,
                ):
                    for oc in range(2):
                        o_psums = [
                            o_ps.tile([128, 512], F32, tag="o", name=f"ops{oc}_{i}")
                            for i in range(8)
                        ]
                        for kt_i in range(NKT):
                            vt = vin.tile([128, 512], DT, tag="vin")
                            nc.sync.dma_start(
                                out=vt,
                                in_=v_s[
                                    kt_i * 128 : (kt_i + 1) * 128,
                                    oc * 512 : (oc + 1) * 512,
                                ],
                            )
                            for qt_i in range(8):
                                nc.tensor.matmul(
                                    o_psums[qt_i][:, :],
                                    et_tiles[kt_i][:, qt_i * 128 : (qt_i + 1) * 128],
                                    vt,
                                    start=(kt_i == 0),
                                    stop=(kt_i == NKT - 1),
                                )
                        for qt_i in range(8):
                            ev = oev.tile([128, 512], F32, tag="oev")
                            nc.vector.tensor_copy(out=ev, in_=o_psums[qt_i][:, :])
                            nc.gpsimd.dma_start(
                                out=out[
                                    qt_i * 128 : (qt_i + 1) * 128,
                                    oc * 512 : (oc + 1) * 512,
                                ],
                                in_=ev,
                            )
    return nc


def _get_program():
    if "nc" not in _CACHE:
        nc = bacc.Bacc("TRN2", target_bir_lowering=False, num_devices=N_CORES)
        _emit(nc)
        nc.compile()
        _CACHE["nc"] = nc
    return _CACHE["nc"]


def kernel(x, Wq, Wk, Wv):
    x = np.asarray(x, dtype=np.float32)
    Wq = np.asarray(Wq, dtype=np.float32)
    Wk = np.asarray(Wk, dtype=np.float32)
    Wv = np.asarray(Wv, dtype=np.float32)

    nc = _get_program()
    ident = np.eye(128, dtype=np.float32)
    in_maps = []
    for c in range(N_CORES):
        b, h = divmod(c, 2)
        if h == 0:
            xr = x[b]
        else:
            xr = np.concatenate([x[b, HALF:], x[b, :HALF]], axis=0)
        in_maps.append(
            {
                "xf": np.ascontiguousarray(xr),
                "wq": Wq,
                "wk": Wk,
                "wv": Wv,
                "ident": ident,
                "ones_in": np.ones((128, 128), dtype=np.float32),
            }
        )
    res = run_bass_kernel_spmd(nc, in_maps, list(range(N_CORES)))
    outp = np.empty((B, S, O), dtype=np.float32)
    for c in range(N_CORES):
        b, h = divmod(c, 2)
        outp[b, h * HALF : (h + 1) * HALF] = res.results[c]["out"]
    return outp


# revision 19
# speedup vs baseline: 14519.5319x; 14519.5319x over previous
"""Trainium2 Bass kernel for single-head attention (no mask).

Reference computation (B=4, S=2048, D=1024):
    q = x @ Wq.T ; k = x @ Wk.T ; v = x @ Wv.T          (per batch)
    out = softmax((q @ k.T) / sqrt(1024)) @ v

Sharding: 8 cores = (batch, query-half). Each core receives its batch's
x with its own query half reordered to the front (attention output is
invariant to a common permutation of the key/value rows), computes K/V
for the full sequence and Q for its 1024 rows, and writes its half of
the output. No collectives: the pair-wise K/V AllGather variant was
evaluated and rejected — the measured-collective cost model prices a
4MB pair gather at ~225us, which dwarfs the ~110us of redundant PE
work it would save.

Scores are computed transposed (S^T[kpos, q]) so the kpos softmax sum
is a ones-matmul (broadcast across partitions) and the PV matmul
consumes exp(S^T) tiles directly as the stationary operand, yielding
output in natural [q, o] layout. Softmax normalization is deferred to
the PSUM eviction of the PV result (per-partition reciprocal scalars
obtained via small PE transposes), keeping it off the critical path.

Matmul operands use float32r (fp32 data, single-pass PE mode, ~4x the
fp32 rate; measured end-to-end relative error ~2e-4).
"""

import numpy as np

import concourse.bass as bass
import concourse.tile as tile
from concourse import bacc, mybir
from concourse.bass_utils import run_bass_kernel_spmd

B, S, D = 4, 2048, 1024
O = 1024  # d_out of each projection
HALF = S // 2  # query rows per core
N_CORES = 8
DT = mybir.dt.float32r
F32 = mybir.dt.float32
SCALE = 1.0 / 32.0  # 1/sqrt(1024)

SB = 256  # s-block (columns of x^T handled per projection block)
NBLK = S // SB  # 8
NQBLK = HALF // SB  # 4 blocks that also produce Q^T
DK = D // 128  # 8 contraction tiles
NOT = O // 128  # 8 output o-tiles
NKT = S // 128  # 16 kpos tiles

_CACHE: dict = {}


def _emit(nc, sfx=""):
    xf = nc.dram_tensor(f"xf{sfx}", [S, D], DT, kind="ExternalInput")
    wq = nc.dram_tensor(f"wq{sfx}", [O, D], DT, kind="ExternalInput")
    wk = nc.dram_tensor(f"wk{sfx}", [O, D], DT, kind="ExternalInput")
    wv = nc.dram_tensor(f"wv{sfx}", [O, D], DT, kind="ExternalInput")
    ident_in = nc.dram_tensor(f"ident{sfx}", [128, 128], DT, kind="ExternalInput")
    ones_in = nc.dram_tensor(f"ones_in{sfx}", [128, 128], DT, kind="ExternalInput")
    out = nc.dram_tensor(f"out{sfx}", [HALF, O], F32, kind="ExternalOutput")
    kt_s = nc.dram_tensor(f"kt_s{sfx}", [O, S], DT)  # K^T scratch
    v_s = nc.dram_tensor(f"v_s{sfx}", [S, O], DT)  # V scratch

    with tile.TileContext(nc) as tc:
        with (
            tc.tile_pool(name=f"{sfx}const", bufs=1) as constp,
            tc.tile_pool(name=f"{sfx}persist", bufs=1) as persist,
        ):
            ident = constp.tile([128, 128], DT, tag="ident")
            nc.sync.dma_start(out=ident, in_=ident_in[:, :])
            ones = constp.tile([128, 128], DT, tag="ones")
            nc.sync.dma_start(out=ones, in_=ones_in[:, :])

            qt = persist.tile([128, NOT, HALF], DT, tag="qt")  # Q^T [o, q]

            # ---------- P0 + P1: projections ----------
            with (
                tc.tile_pool(name=f"{sfx}wt", bufs=1) as wtp,
                tc.tile_pool(name=f"{sfx}wnat", bufs=3) as wnat,
                tc.tile_pool(name=f"{sfx}xin", bufs=3) as xin,
                tc.tile_pool(name=f"{sfx}xt", bufs=2) as xtp,
                tc.tile_pool(name=f"{sfx}kqev", bufs=3) as kqev,
                tc.tile_pool(name=f"{sfx}vev", bufs=3) as vev,
                tc.tile_pool(name=f"{sfx}tp_ps", bufs=3, space="PSUM") as tp_ps,
                tc.tile_pool(name=f"{sfx}kq_ps", bufs=3, space="PSUM") as kq_ps,
                tc.tile_pool(name=f"{sfx}v_ps", bufs=2, space="PSUM") as v_ps,
            ):
                # W^T tiles: [128(d within tile), DK index, O]
                wts = {}
                for name, wsrc in (("wtk", wk), ("wtv", wv), ("wtq", wq)):
                    wt = wtp.tile([128, DK, O], DT, tag=name, name=f"{name}{sfx}")
                    wts[name] = wt
                    for ot in range(NOT):
                        wn = wnat.tile([128, D], DT, tag="wnat")
                        nc.sync.dma_start(
                            out=wn, in_=wsrc[ot * 128 : (ot + 1) * 128, :]
                        )
                        for dk in range(0, DK, 2):
                            ps = tp_ps.tile([128, 256], DT, tag="tp")
                            nc.tensor.transpose(
                                ps[:, 0:128], wn[:, dk * 128 : (dk + 1) * 128], ident
                            )
                            nc.tensor.transpose(
                                ps[:, 128:256],
                                wn[:, (dk + 1) * 128 : (dk + 2) * 128],
                                ident,
                            )
                            nc.vector.tensor_copy(
                                out=wt[:, dk, ot * 128 : (ot + 1) * 128],
                                in_=ps[:, 0:128],
                            )
                            nc.vector.tensor_copy(
                                out=wt[:, dk + 1, ot * 128 : (ot + 1) * 128],
                                in_=ps[:, 128:256],
                            )

                wtk, wtv, wtq = wts["wtk"], wts["wtv"], wts["wtq"]

                def load_and_transpose(blk):
                    # load 2 natural x tiles, transpose to xt_blk [128, DK, SB]
                    xt_blk = xtp.tile([128, DK, SB], DT, tag="xt", name=f"xt{sfx}_{blk}")
                    xtiles = []
                    for st in range(2):
                        xn = xin.tile([128, D], DT, tag="xin", name=f"xin{sfx}_{blk}_{st}")
                        nc.sync.dma_start(
                            out=xn,
                            in_=xf[blk * SB + st * 128 : blk * SB + (st + 1) * 128, :],
                        )
                        xtiles.append(xn)
                    for dk in range(DK):
                        ps = tp_ps.tile([128, 256], DT, tag="tp", name=f"tp{sfx}_{blk}_{dk}")
                        nc.tensor.transpose(
                            ps[:, 0:128], xtiles[0][:, dk * 128 : (dk + 1) * 128], ident
                        )
                        nc.tensor.transpose(
                            ps[:, 128:256],
                            xtiles[1][:, dk * 128 : (dk + 1) * 128],
                            ident,
                        )
                        nc.scalar.copy(out=xt_blk[:, dk, :], in_=ps[:, :])
                    return xt_blk

                next_xt = load_and_transpose(0)
                for blk in range(NBLK):
                    xt_blk = next_xt
                    # software pipeline: transpose the next block before this
                    # block's matmuls so the PE never waits on the copies
                    if blk + 1 < NBLK:
                        next_xt = load_and_transpose(blk + 1)

                    # K^T[:, blk]
                    for ot in range(NOT):
                        ps = kq_ps.tile([128, SB], F32, tag="kq")
                        for dk in range(DK):
                            nc.tensor.matmul(
                                ps[:, :],
                                wtk[:, dk, ot * 128 : (ot + 1) * 128],
                                xt_blk[:, dk, :],
                                start=(dk == 0),
                                stop=(dk == DK - 1),
                            )
                        ev = kqev.tile([128, SB], DT, tag="kqev")
                        nc.vector.tensor_copy(out=ev, in_=ps[:, :])
                        nc.gpsimd.dma_start(
                            out=kt_s[
                                ot * 128 : (ot + 1) * 128, blk * SB : (blk + 1) * SB
                            ],
                            in_=ev,
                        )

                    # V[blk, :]
                    for st in range(2):
                        for oc in range(2):
                            ps = v_ps.tile([128, 512], F32, tag="v")
                            for dk in range(DK):
                                nc.tensor.matmul(
                                    ps[:, :],
                                    xt_blk[:, dk, st * 128 : (st + 1) * 128],
                                    wtv[:, dk, oc * 512 : (oc + 1) * 512],
                                    start=(dk == 0),
                                    stop=(dk == DK - 1),
                                )
                            ev = vev.tile([128, 512], DT, tag="vev")
                            nc.vector.tensor_copy(out=ev, in_=ps[:, :])
                            nc.gpsimd.dma_start(
                                out=v_s[
                                    blk * SB + st * 128 : blk * SB + (st + 1) * 128,
                                    oc * 512 : (oc + 1) * 512,
                                ],
                                in_=ev,
                            )

                    # Q^T[:, blk] (first NQBLK blocks hold this core's queries)
                    if blk < NQBLK:
                        for ot in range(NOT):
                            ps = kq_ps.tile([128, SB], F32, tag="kq")
                            for dk in range(DK):
                                nc.tensor.matmul(
                                    ps[:, :],
                                    wtq[:, dk, ot * 128 : (ot + 1) * 128],
                                    xt_blk[:, dk, :],
                                    start=(dk == 0),
                                    stop=(dk == DK - 1),
                                )
                            nc.vector.tensor_copy(
                                out=qt[:, ot, blk * SB : (blk + 1) * SB], in_=ps[:, :]
                            )

            # ---------- P2: scores^T + exp + rowsums ----------
            with tc.tile_pool(name=f"{sfx}et", bufs=1) as etp:
                et_tiles = [
                    etp.tile([128, HALF], DT, tag=f"et{i}", name=f"et{sfx}_{i}")
                    for i in range(NKT)
                ]
                recip_col = etp.tile([128, 8], F32, tag="recip_col", name=f"recip_col{sfx}")
                with (
                    tc.tile_pool(name=f"{sfx}ktin", bufs=4) as ktin,
                    tc.tile_pool(name=f"{sfx}rsb", bufs=1) as rsb,
                    tc.tile_pool(name=f"{sfx}s_ps", bufs=2, space="PSUM") as s_ps,
                    tc.tile_pool(name=f"{sfx}rs_ps", bufs=1, space="PSUM") as rs_ps,
                    tc.tile_pool(name=f"{sfx}rc_ps", bufs=2, space="PSUM") as rc_ps,
                ):
                    ps_rs = rs_ps.tile([128, HALF], F32, tag="rs")

                    for kt_i in range(NKT):
                        ktt = ktin.tile([128, NOT, 128], DT, tag="ktin")
                        nc.sync.dma_start(
                            out=ktt,
                            in_=kt_s[:, kt_i * 128 : (kt_i + 1) * 128].rearrange(
                                "(a p) f -> p a f", p=128
                            ),
                        )
                        ps = s_ps.tile([128, HALF], F32, tag="s")
                        for qc in range(2):
                            for ok in range(NOT):
                                nc.tensor.matmul(
                                    ps[:, qc * 512 : (qc + 1) * 512],
                                    ktt[:, ok, :],
                                    qt[:, ok, qc * 512 : (qc + 1) * 512],
                                    start=(ok == 0),
                                    stop=(ok == NOT - 1),
                                )
                        nc.scalar.activation(
                            out=et_tiles[kt_i],
                            in_=ps[:, :],
                            func=mybir.ActivationFunctionType.Exp,
                            scale=SCALE,
                        )
                        for qc in range(2):
                            nc.tensor.matmul(
                                ps_rs[:, qc * 512 : (qc + 1) * 512],
                                ones,
                                et_tiles[kt_i][:, qc * 512 : (qc + 1) * 512],
                                start=(kt_i == 0),
                                stop=(kt_i == NKT - 1),
                            )

                    # 1/rowsum, transposed into per-q-tile column vectors so
                    # normalization folds into the P3 eviction.
                    recip_b = rsb.tile([128, HALF], DT, tag="recip_b")
                    with nc.allow_low_precision(reason="f32r is bitwise fp32"):
                        nc.vector.reciprocal(out=recip_b, in_=ps_rs[:, :])
                    for t in range(8):
                        tps = rc_ps.tile([128, 128], DT, tag="rc", name=f"rc{sfx}_{t}")
                        nc.tensor.transpose(
                            tps[:, :], recip_b[:, t * 128 : (t + 1) * 128], ident
                        )
                        nc.vector.tensor_copy(
                            out=recip_col[:, t : t + 1], in_=tps[:, 0:1]
                        )

                # ---------- P3: out = (E^T)^T @ V ----------
                with (
                    tc.tile_pool(name=f"{sfx}vin", bufs=4) as vin,
                    tc.tile_pool(name=f"{sfx}oev", bufs=3) as oev,
                    tc.tile_pool(name=f"{sfx}o_ps", bufs=8, space="PSUM") as o_ps,
                ):
                    # 4 chunks of (q-half, o-half): 4 PSUM banks each, so one
                    # chunk's accumulation overlaps the previous chunk's
                    # eviction + output DMA.
                    for oc in range(2):
                        for qh in range(2):
                            o_psums = [
                                o_ps.tile(
                                    [128, 512], F32, tag="o", name=f"ops{sfx}{oc}{qh}{i}"
                                )
                                for i in range(4)
                            ]
                            for kt_i in range(NKT):
                                vt = vin.tile(
                                    [128, 512], DT, tag="vin", name=f"v{sfx}_{oc}{qh}{kt_i}"
                                )
                                nc.sync.dma_start(
                                    out=vt,
                                    in_=v_s[
                                        kt_i * 128 : (kt_i + 1) * 128,
                                        oc * 512 : (oc + 1) * 512,
                                    ],
                                )
                                for j in range(4):
                                    qt_i = qh * 4 + j
                                    nc.tensor.matmul(
                                        o_psums[j][:, :],
                                        et_tiles[kt_i][
                                            :, qt_i * 128 : (qt_i + 1) * 128
                                        ],
                                        vt,
                                        start=(kt_i == 0),
                                        stop=(kt_i == NKT - 1),
                                    )
                            for j in range(4):
                                qt_i = qh * 4 + j
                                ev = oev.tile([128, 512], F32, tag="oev")
                                nc.vector.tensor_scalar_mul(
                                    out=ev,
                                    in0=o_psums[j][:, :],
                                    scalar1=recip_col[:, qt_i : qt_i + 1],
                                )
                                nc.gpsimd.dma_start(
                                    out=out[
                                        qt_i * 128 : (qt_i + 1) * 128,
                                        oc * 512 : (oc + 1) * 512,
                                    ],
                                    in_=ev,
                                )
    return nc


def _get_program():
    if "nc" not in _CACHE:
        nc = bacc.Bacc("TRN2", target_bir_lowering=False, num_devices=N_CORES)
        _emit(nc)
        nc.compile()
        _CACHE["nc"] = nc
    return _CACHE["nc"]


def kernel(x, Wq, Wk, Wv):
    x = np.asarray(x, dtype=np.float32)
    Wq = np.asarray(Wq, dtype=np.float32)
    Wk = np.asarray(Wk, dtype=np.float32)
    Wv = np.asarray(Wv, dtype=np.float32)

    nc = _get_program()
    ident = np.eye(128, dtype=np.float32)
    ones = np.ones((128, 128), dtype=np.float32)
    in_maps = []
    for c in range(N_CORES):
        b, h = divmod(c, 2)
        if h == 0:
            xr = x[b]
        else:
            xr = np.concatenate([x[b, HALF:], x[b, :HALF]], axis=0)
        in_maps.append(
            {
                "xf": np.ascontiguousarray(xr),
                "wq": Wq,
                "wk": Wk,
                "wv": Wv,
                "ident": ident,
                "ones_in": ones,
            }
        )
    res = run_bass_kernel_spmd(nc, in_maps, list(range(N_CORES)))
    outp = np.empty((B, S, O), dtype=np.float32)
    for c in range(N_CORES):
        b, h = divmod(c, 2)
        outp[b, h * HALF : (h + 1) * HALF] = res.results[c]["out"]
    return outp
